# revision 1
# baseline (speedup 1.0000x reference)
"""Causal attention with ALiBi + conv projections, sharded over 8 trn2 cores.

Sharding: core c handles batch b=c//4 and head pair (c%4, c%4+4).
Each core computes LayerNorm + projections for its batch, attention for its
two heads, and a partial output projection over its 128 channels; the host
sums the 4 partials per batch.

All cores run an identical instruction stream (SPMD); per-head differences
(ALiBi slope, block-skip extent, diagonal mask) enter only through input
data: an E-table for the "light" head (h<4, steep slopes) and exp-bias
columns for the "heavy" head (h>=4, shallow slopes).
"""

import math

import ml_dtypes
import numpy as np

import concourse.bass as bass
from concourse import bacc
import concourse.tile as tile
from concourse import mybir
from concourse.bass_utils import run_bass_kernel_spmd

B, N, DIM = 2, 2048, 512
HEADS, DH = 8, 64
P = 128
NT = N // P            # 16 n-tiles
NC4 = N // 512         # 4 column chunks
KMAX_L = 14            # light head: j-tiles kept per i-block (covers h3; E-table
                       # data zeroes the unused tail for h0-h2)
KMAX_H = 16            # heavy head: no skipping
SW = 1024              # strip width (queries per softmax strip)
NSTRIP = N // SW
NB = SW // P
EC = (KMAX_L - 1) * P + SW + P  # E-table columns: 13*128 + 1024 + 128
F32 = mybir.dt.float32
BF16 = mybir.dt.bfloat16

_SLOPES = None


def _slopes():
    global _SLOPES
    if _SLOPES is None:
        start = 2 ** (-(2 ** (-(math.log2(HEADS) - 3))))
        _SLOPES = [start * start**i for i in range(HEADS)]
    return _SLOPES


def _units(kmax):
    for s in range(NSTRIP):
        ib0 = NB * s
        for jb in range(max(0, ib0 - kmax + 1), ib0 + NB):
            lo = max(ib0, jb)
            hi = min(ib0 + NB - 1, jb + kmax - 1)
            L = P * (lo - ib0)
            W = P * (hi - lo + 1)
            e_off = P * max(0, ib0 - jb)
            yield s, jb, L, W, e_off


def _biasH_cols():
    """Enumerate (strip, jb) pairs for the heavy head."""
    return [(u[0], u[1]) for u in _units(KMAX_H)]


def _light_units():
    return list(_units(KMAX_L))


def _heavy_units():
    return [u[:4] for u in _units(KMAX_H)]


def _build_program():
    nc = bacc.Bacc()
    f = F32
    xb = nc.declare_dram_parameter("xb", [N, DIM], f, False)
    wqT = nc.declare_dram_parameter("wqT", [DIM, P], BF16, False)
    wkT = nc.declare_dram_parameter("wkT", [DIM, P], BF16, False)
    wvT = nc.declare_dram_parameter("wvT", [DIM, P], BF16, False)
    convq = nc.declare_dram_parameter("convq", [P, 3 * P], BF16, False)
    convk = nc.declare_dram_parameter("convk", [P, 3 * P], BF16, False)
    convv = nc.declare_dram_parameter("convv", [P, 3 * P], BF16, False)
    w_oL = nc.declare_dram_parameter("w_oL", [DH, DIM], BF16, False)
    w_oH = nc.declare_dram_parameter("w_oH", [DH, DIM], BF16, False)
    ident = nc.declare_dram_parameter("ident", [P, P], BF16, False)
    tri01 = nc.declare_dram_parameter("tri01", [P, P], BF16, False)
    etab = nc.declare_dram_parameter("etab", [P, EC], BF16, False)
    biasH = nc.declare_dram_parameter("biasH", [P, len(_biasH_cols())], f, False)
    out = nc.declare_dram_parameter("out", [N, DIM], f, True)

    with tile.TileContext(nc) as tc:
        with (
            tc.tile_pool(name="singles", bufs=1) as singles,
            tc.tile_pool(name="xnpool", bufs=4) as xnpool,
            tc.tile_pool(name="stat", bufs=4) as stat,
            tc.tile_pool(name="ppool", bufs=14) as ppool,
            tc.tile_pool(name="ypad", bufs=2) as ypad_pool,
            tc.tile_pool(name="opool", bufs=3) as opool,
            tc.tile_pool(name="ps", bufs=2, space="PSUM") as ps,
            tc.tile_pool(name="psA", bufs=2, space="PSUM") as psA,
        ):
            # ---- input x first (sync queue), identity early for transposes ----
            x_sb = singles.tile([P, NT, DIM], f, tag="xsb")
            for t in range(3):
                nc.sync.dma_start(out=x_sb[:, t, :], in_=xb[t * P : (t + 1) * P, :])
            ident_dma = singles.tile([P, P], BF16, tag="identd")
            nc.sync.dma_start(out=ident_dma, in_=ident[:, :])
            # bounce through DVE so PE transposes wait on a single engine sem
            ident_sb = singles.tile([P, P], BF16, tag="ident")
            nc.vector.tensor_copy(out=ident_sb, in_=ident_dma)
            for t in range(3, NT):
                nc.sync.dma_start(out=x_sb[:, t, :], in_=xb[t * P : (t + 1) * P, :])

            wT_sb = {}
            for nm, t in (("q", wqT), ("k", wkT), ("v", wvT)):
                w = singles.tile([P, 4, P], BF16, tag=f"w{nm}T")
                nc.sync.dma_start(
                    out=w, in_=t.rearrange("(t p) c -> p t c", p=P)
                )
                wT_sb[nm] = w
            conv_sb = {}
            for nm, t in (("q", convq), ("k", convk), ("v", convv)):
                w = singles.tile([P, 3 * P], BF16, tag=f"conv{nm}")
                nc.sync.dma_start(out=w, in_=t[:, :])
                conv_sb[nm] = w

            # late-needed constants go on the gpsimd queue (parallel FIFO)
            tri_sb = singles.tile([P, P], BF16, tag="tri")
            nc.gpsimd.dma_start(out=tri_sb, in_=tri01[:, :])
            etab_sb = singles.tile([P, EC], BF16, tag="etab")
            nc.gpsimd.dma_start(out=etab_sb, in_=etab[:, :])
            biasH_sb = singles.tile([P, len(_biasH_cols())], F32, tag="biasH")
            nc.gpsimd.dma_start(out=biasH_sb, in_=biasH[:, :])
            woL_sb = singles.tile([DH, DIM], BF16, tag="woL")
            nc.gpsimd.dma_start(out=woL_sb, in_=w_oL[:, :])
            woH_sb = singles.tile([DH, DIM], BF16, tag="woH")
            nc.gpsimd.dma_start(out=woH_sb, in_=w_oH[:, :])
            eps_sb = singles.tile([P, 1], f, tag="eps")
            nc.vector.memset(eps_sb, 1e-5)
            ones_sb = singles.tile([65, DH], F32, tag="ones")
            nc.vector.memset(ones_sb, 1.0)

            # ---- LayerNorm + transpose -> xnT (4 tiles [128, N]) ----
            xnT = [
                singles.tile([P, N], BF16, tag=f"xnT{dt}", name=f"xnT{dt}")
                for dt in range(4)
            ]
            for t in range(NT):
                x_t = x_sb[:, t, :]
                st = stat.tile([P, 6], F32, tag="bnst")
                nc.vector.bn_stats(out=st, in_=x_t)
                mv = stat.tile([P, 2], F32, tag="bnmv")
                nc.vector.bn_aggr(out=mv, in_=st)
                std = stat.tile([P, 1], F32, tag="std")
                nc.scalar.activation(
                    out=std, in_=mv[:, 1:2],
                    func=mybir.ActivationFunctionType.Sqrt,
                    bias=eps_sb, scale=1.0,
                )
                rstd = stat.tile([P, 1], F32, tag="rstd")
                nc.vector.reciprocal(out=rstd, in_=std)
                xn_t = xnpool.tile([P, DIM], BF16, tag="xn")
                nc.vector.tensor_scalar(
                    out=xn_t, in0=x_t,
                    scalar1=mv[:, 0:1], scalar2=rstd,
                    op0=mybir.AluOpType.subtract, op1=mybir.AluOpType.mult,
                )
                for dt in range(4):
                    tp = ps.tile([P, 1024], BF16, tag="w", name="mmt")[:, 0:P]
                    nc.tensor.transpose(tp, xn_t[:, dt * P : (dt + 1) * P], ident_sb)
                    nc.vector.tensor_copy(
                        out=xnT[dt][:, t * P : (t + 1) * P], in_=tp
                    )

            # ---- projections: pointwise + depthwise conv (via diag matmuls) ----
            y = {}
            for nm in ("q", "k", "v"):
                pad = ypad_pool.tile([P, N + 4], BF16, tag="ypad")
                nc.vector.memset(pad[:, 0:2], 0.0)
                for c4 in range(NC4):
                    yp_ps = ps.tile([P, 512], F32, tag="w")
                    for dt in range(4):
                        nc.tensor.matmul(
                            yp_ps, wT_sb[nm][:, dt, :],
                            xnT[dt][:, c4 * 512 : (c4 + 1) * 512],
                            start=(dt == 0), stop=(dt == 3),
                        )
                    nc.vector.tensor_copy(
                        out=pad[:, 2 + c4 * 512 : 2 + (c4 + 1) * 512], in_=yp_ps
                    )
                y_nm = singles.tile([P, N], BF16, tag=f"y{nm}")
                for c4 in range(NC4):
                    yc_ps = ps.tile([P, 512], F32, tag="w")
                    for t3 in range(3):
                        nc.tensor.matmul(
                            yc_ps, conv_sb[nm][:, t3 * P : (t3 + 1) * P],
                            pad[:, t3 + c4 * 512 : t3 + c4 * 512 + 512],
                            start=(t3 == 0), stop=(t3 == 2),
                        )
                    nc.vector.tensor_copy(
                        out=y_nm[:, c4 * 512 : (c4 + 1) * 512], in_=yc_ps
                    )
                y[nm] = y_nm

            # ---- V transposed into per-j-tile [v | ones] tiles ----
            vaug = {0: [], 1: []}
            for hx in range(2):
                r0 = hx * DH
                for jb in range(NT):
                    vp = ps.tile([P, 1024], BF16, tag="w", name="mmv")[:, 0:DH]
                    nc.tensor.transpose(
                        vp,
                        y["v"][r0 : r0 + DH, jb * P : (jb + 1) * P],
                        ident_sb[r0 : r0 + DH, r0 : r0 + DH],
                    )
                    va = singles.tile([P, DH + 1], BF16, tag=f"vaug{hx}_{jb}")
                    nc.vector.tensor_copy(out=va[:, 0:DH], in_=vp)
                    nc.vector.memset(va[:, DH : DH + 1], 1.0)
                    vaug[hx].append(va)

            # ---- attention ----
            aTn = {
                0: singles.tile([DH, N], BF16, tag="aTnL", name="aTnL"),
                1: singles.tile([DH, N], BF16, tag="aTnH", name="aTnH"),
            }
            bcols = {sj: i for i, sj in enumerate(_biasH_cols())}

            def pv_chunks(L, W):
                """Split [L, L+W) at PSUM f32 bank boundaries (512 cols)."""
                cks, c = [], L
                while c < L + W:
                    c1 = min(L + W, (c // 512 + 1) * 512)
                    cks.append((c, c1))
                    c = c1
                return cks

            def emit_pair(s, units_by_hx, i):
                """Emit unit i of both heads (chunked QK + exp). Returns PV items."""
                items = []
                for hx, units in units_by_hx.items():
                    if i >= len(units):
                        continue
                    _, jb, L, W, e_off = units[i]
                    for (c0, c1) in pv_chunks(L, W):
                        r0 = hx * DH
                        cw = c1 - c0
                        sT = ps.tile([P, 512], F32, tag="sT", name="sT")
                        nc.tensor.matmul(
                            sT[:, 0:cw],
                            y["k"][r0 : r0 + DH, jb * P : (jb + 1) * P],
                            y["q"][r0 : r0 + DH, s * SW + c0 : s * SW + c1],
                            start=True, stop=True,
                        )
                        items.append(emit_exp(s, hx, jb, L, e_off, c0, c1, sT))
                return items

            def emit_exp(s, hx, jb, L, e_off, c0, c1, sT):
                cw = c1 - c0
                p_sb = ppool.tile([P, 512], BF16, tag="p")
                if hx == 0:
                    nc.scalar.activation(
                        out=p_sb[:, 0:cw], in_=sT[:, 0:cw],
                        func=mybir.ActivationFunctionType.Exp,
                        bias=0.0, scale=1.0,
                    )
                    eo = e_off + (c0 - L)
                    nc.vector.tensor_mul(
                        p_sb[:, 0:cw], p_sb[:, 0:cw],
                        etab_sb[:, eo : eo + cw],
                    )
                else:
                    col = bcols[(s, jb)]
                    nc.scalar.activation(
                        out=p_sb[:, 0:cw], in_=sT[:, 0:cw],
                        func=mybir.ActivationFunctionType.Exp,
                        bias=biasH_sb[:, col : col + 1], scale=1.0,
                    )
                    if jb >= NB * s and c0 == L:
                        nc.vector.tensor_mul(
                            p_sb[:, 0:P], p_sb[:, 0:P], tri_sb
                        )
                return (hx, (jb, c0, c1, p_sb))

            def emit_pv(hx, A, item, bank_first, bank_last):
                jb, c0, c1, p_sb = item
                bank = c0 // 512
                last = bank_last[bank] == (jb, c0)
                nc.tensor.matmul(
                    A[:, c0:c1], vaug[hx][jb], p_sb[:, 0 : c1 - c0],
                    start=bank_first[bank] == (jb, c0),
                    stop=last,
                )
                return bank if last else None

            def rinv_bank(A, rowsum, rinv, rg):
                sl = slice(rg * 512, (rg + 1) * 512)
                nc.vector.reciprocal(out=rinv[64:65, sl], in_=A[64:65, sl])

            def norm_chunk(hx, s, A, rinv, rg):
                Rb_ps = ps.tile([P, 512], F32, tag="w", name="mmr")[0:DH, :]
                nc.tensor.matmul(
                    Rb_ps, ones_sb[64:65, :],
                    rinv[64:65, rg * 512 : (rg + 1) * 512],
                    start=True, stop=True,
                )
                Rb_sb = ppool.tile([DH, 512], F32, tag="Rbsb")
                nc.vector.tensor_copy(out=Rb_sb, in_=Rb_ps)
                nc.vector.tensor_mul(
                    aTn[hx][:, s * SW + rg * 512 : s * SW + (rg + 1) * 512],
                    A[0:DH, rg * 512 : (rg + 1) * 512], Rb_sb,
                )

            def outproj_block(t):
                o_ps = ps.tile([P, 512], F32, tag="w")
                nc.tensor.matmul(
                    o_ps, aTn[0][:, t * P : (t + 1) * P], woL_sb,
                    start=True, stop=False,
                )
                nc.tensor.matmul(
                    o_ps, aTn[1][:, t * P : (t + 1) * P], woH_sb,
                    start=False, stop=True,
                )
                o_sb = opool.tile([P, DIM], f, tag="osb")
                nc.vector.tensor_copy(out=o_sb, in_=o_ps)
                nc.sync.dma_start(out=out[t * P : (t + 1) * P, :], in_=o_sb)

            lu = _light_units()
            hu = [(u[0], u[1], u[2], u[3], 0) for u in _heavy_units()]
            LAG = 6
            deferred = []

            def pop_deferred():
                if deferred:
                    deferred.pop(0)()

            for s in range(NSTRIP):
                ordered, A_t, first, last = {}, {}, {}, {}
                rowsum_t, rinv_t = {}, {}
                for hx, units in ((0, lu), (1, hu)):
                    us = sorted(
                        [u for u in units if u[0] == s],
                        key=lambda u: (u[3] != SW, u[1]),
                    )
                    ordered[hx] = us
                    A_t[hx] = psA.tile([65, SW], F32, tag="A", name="A")
                    rowsum_t[hx] = stat.tile([65, SW], F32, tag="rowsum", name="rowsum")
                    rinv_t[hx] = stat.tile([65, SW], F32, tag="rinv", name="rinv")
                    bf, bl = {}, {}
                    for u in us:
                        for (c0, c1) in pv_chunks(u[2], u[3]):
                            bank = c0 // 512
                            bf.setdefault(bank, (u[1], c0))
                            bl[bank] = (u[1], c0)
                    first[hx] = bf
                    last[hx] = bl
                # norm-chunk/outproj release bookkeeping
                normed = {0: set(), 1: set()}

                def bank_done(hx, bank, s=s):
                    rinv_bank(A_t[hx], rowsum_t[hx], rinv_t[hx], bank)
                    deferred.append(
                        lambda hx=hx, s=s, A=A_t[hx], rv=rinv_t[hx], rg=bank:
                            norm_chunk(hx, s, A, rv, rg)
                    )
                    normed[hx].add(bank)
                    if bank in normed[0] and bank in normed[1]:
                        for t in range(s * NB + bank * 4, s * NB + bank * 4 + 4):
                            deferred.append(lambda t=t: outproj_block(t))

                queue = []
                nu = max(len(ordered[0]), len(ordered[1]))
                for i in range(nu):
                    if i >= 2:
                        pop_deferred()
                    for hi in emit_pair(s, ordered, i):
                        queue.append(hi)
                    while len(queue) > 2 * LAG:
                        qhx, qitem = queue.pop(0)
                        done = emit_pv(qhx, A_t[qhx], qitem, first[qhx], last[qhx])
                        if done is not None:
                            bank_done(qhx, done)
                for qhx, qitem in queue:
                    done = emit_pv(qhx, A_t[qhx], qitem, first[qhx], last[qhx])
                    if done is not None:
                        bank_done(qhx, done)
            while deferred:
                pop_deferred()

    if not nc.is_finalized():
        nc.finalize()
    return nc


_CACHE = {}


def _get_program():
    if "nc" not in _CACHE:
        _CACHE["nc"] = _build_program()
    return _CACHE["nc"]


def _host_inputs(inputs, c):
    """Build the per-core input map (all float32, layout-prepped)."""
    slopes = _slopes()
    b, qh = c // 4, c % 4
    hL, hH = qh, qh + 4
    ch = np.r_[hL * DH : hL * DH + DH, hH * DH : hH * DH + DH]
    scale = DH ** -0.5
    f4 = np.float32

    x = np.ascontiguousarray(inputs["x"][b], dtype=f4)
    wq = (inputs["wq1"][ch] * scale).astype(f4)
    wk = inputs["wk1"][ch].astype(f4)
    wv = inputs["wv1"][ch].astype(f4)

    def diag3(wd):
        out = np.zeros((P, 3 * P), f4)
        for t in range(3):
            out[:, t * P : (t + 1) * P][np.arange(P), np.arange(P)] = wd[:, t]
        return out

    jj = np.arange(P)[:, None]
    m = np.arange(EC)[None, :]
    sl = slopes[hL]
    with np.errstate(under="ignore"):
        etab = np.exp(sl * (jj - m)).astype(f4)
    etab[:, :P] *= (jj <= m[:, :P])

    bc = _biasH_cols()
    slh = slopes[hH]
    biasH = np.zeros((P, len(bc)), f4)
    for i, (s, jb) in enumerate(bc):
        r = s * SW + SW - 1
        biasH[:, i] = slh * (P * jb + jj[:, 0] - r)

    bf = ml_dtypes.bfloat16
    return {
        "xb": x,
        "wqT": np.ascontiguousarray(wq.T).astype(bf),
        "wkT": np.ascontiguousarray(wk.T).astype(bf),
        "wvT": np.ascontiguousarray(wv.T).astype(bf),
        "convq": diag3(inputs["wqd"][ch].astype(f4)).astype(bf),
        "convk": diag3(inputs["wkd"][ch].astype(f4)).astype(bf),
        "convv": diag3(inputs["wvd"][ch].astype(f4)).astype(bf),
        "w_oL": np.ascontiguousarray(inputs["wout"][:, ch[:DH]].T.astype(f4)).astype(bf),
        "w_oH": np.ascontiguousarray(inputs["wout"][:, ch[DH:]].T.astype(f4)).astype(bf),
        "ident": np.eye(P, dtype=f4).astype(bf),
        "tri01": (jj <= np.arange(P)[None, :]).astype(f4).astype(bf),
        "etab": etab.astype(bf),
        "biasH": biasH,
    }


def kernel(**inputs):
    nc = _get_program()
    in_maps = [_host_inputs(inputs, c) for c in range(8)]
    res = run_bass_kernel_spmd(nc, in_maps, core_ids=list(range(8)))
    out = np.zeros((B, N, DIM), np.float32)
    for c in range(8):
        out[c // 4] += res.results[c]["out"]
    return out

